# revision 71
# baseline (speedup 1.0000x reference)
"""AdaConv2D (instance-norm -> grouped 3x3 conv -> grouped 1x1 conv -> bias) on 8 TRN2 cores.

Strategy (pure data parallel: batch dim sharded, 1 sample per NeuronCore, no
collectives; measured ~237-245us HW exec, rel err ~2.2e-3 vs the f32 reference):

  Host-side prep (inside kernel(), before launch):
  - Fuse the grouped 1x1 conv into the grouped 3x3 conv: both share the same
    4-channel group partition, so eff[g,j,i,kh,kw] = sum_m pw[g,j,m]*dw[g,m,i,kh,kw]
    gives ONE effective grouped 3x3 conv.
  - Pack eff into block-diagonal 32x32 bf16 lhsT tiles (8 groups of 4x4 per tile),
    one per (channel sub-chunk r, tap); x is converted to bf16 (rel-err budget
    2e-2 >> bf16 rounding; halves input DMA traffic).

  Device, per 128-channel chunk (4 chunks/sample), software-pipelined 3 deep with
  STRICT engine specialization (each engine executes its scheduled queue in order,
  so cross-chunk work must never queue behind blocking predecessors):
  - Sync (HWDGE): input DMA, 4 slices/chunk, issued two chunks ahead; output DMA.
  - ACT: instance-norm statistics via Copy/Square activations with accum_out
    (sum and sum-of-squares per slice, zero DVE cost), issued two chunks ahead.
  - GpSimd: halo memsets, the mean/var/rstd chain (tensor-tensor ops + 3 ACT
    transcendentals: rstd = exp(-ln(sqrt(var*N/(N-1))+eps))), and the normalize
    pass (8 pieces, tensor_scalar x*rstd + (-mean*rstd)), one chunk ahead.
  - DVE: PSUM eviction ONLY (psum + per-partition bias -> bf16 staging tile);
    anything else on this queue stalls TensorE via PSUM back-pressure.
  - TensorE: conv as 4 concurrent 64x64 tile_position matmuls per span =
    2 channel sub-chunk PAIRS (row groups; contraction = 64 channels,
    16 groups block-diag) x 2 spatial halves (col groups).  9 taps = shifted
    APs on a row-padded SBUF layout (1 zero halo row above/below, rows of 128
    contiguous), accumulated in PSUM (start on the first dw=0 tap).  W-edge
    padding is done by SHRINKING the free dim of dw=+-1 taps (edge output
    columns simply don't receive those taps).  Spatial tile of col group C at
    span q is t = 16C + q, so each partition half owns a contiguous spatial
    half, giving 8KB-contiguous output DMA runs.  The PE is instruction-issue
    bound, so fewer/bigger tiles beat 16x 32x32 (measured 238us -> 204us);
    LDWEIGHTS is 1:1 with matmuls (walrus ldw-opt crashes, N=1024 moving
    operand is ISA-rejected for the one-PSUM-bank limit).
  - Output staged in bf16 (halves output traffic), stored per quarter-drain;
    host upcasts to f32.
"""
import os
import sys
import numpy as np
import ml_dtypes

if "/opt/trn_rl_repo" not in sys.path:
    sys.path.insert(0, "/opt/trn_rl_repo")

B, C, H, W = 8, 512, 128, 128
HW = H * W            # 16384
NCH = 4               # 128-channel chunks per sample
NTAP = 9
ROWS_PAD = H + 2      # 130 rows of 128 in padded SBUF layout
PADF = ROWS_PAD * W   # 16640 elems per partition
EPS = 1e-7
# taps ordered so the first three are dw=0 (full-width writes -> correct PSUM init)
TAPS = [(0, 1), (1, 1), (2, 1), (0, 0), (1, 0), (2, 0), (0, 2), (1, 2), (2, 2)]

_CACHE = {}


def _build_program():
    import concourse.bass as bass
    import concourse.tile as tile
    from concourse import bacc, mybir

    f32 = mybir.dt.float32
    bf16 = mybir.dt.bfloat16
    MULT = mybir.AluOpType.mult
    ADD = mybir.AluOpType.add
    SUB = mybir.AluOpType.subtract
    IDENT = mybir.ActivationFunctionType.Identity
    nc = bacc.Bacc("TRN2", target_bir_lowering=False, debug=False,
                   enable_asserts=False, num_devices=8)

    x_d = nc.dram_tensor("x", [C, HW], bf16, kind="ExternalInput")
    w_d = nc.dram_tensor("w", [128, NCH * NTAP * 64], bf16, kind="ExternalInput")
    b_d = nc.dram_tensor("bias", [128, 8], f32, kind="ExternalInput")
    out_d = nc.dram_tensor("out", [C, HW], bf16, kind="ExternalOutput")

    # store view: [cc, Ch(spatial half), hh(drain half), p, R, e(4096)]
    out_v = out_d[:].rearrange("(a R p) (Ch hh e) -> a Ch hh p R e", a=NCH, R=2,
                               p=64, Ch=2, hh=2, e=4096)

    with tile.TileContext(nc) as tc:
        with (
            tc.tile_pool(name="xpool", bufs=3) as xpool,
            tc.tile_pool(name="wpool", bufs=1) as wpool,
            tc.tile_pool(name="spool", bufs=3) as spool,
            tc.tile_pool(name="opool", bufs=2) as opool,
            tc.tile_pool(name="psum", bufs=8, space=bass.MemorySpace.PSUM) as pspool,
        ):
            w_sb = wpool.tile([128, NCH * NTAP * 64], bf16)
            nc.sync.dma_start(w_sb[:], w_d[:])
            bias_sb = wpool.tile([128, 8], f32)
            nc.sync.dma_start(bias_sb[:], b_d[:])
            trash0 = wpool.tile([128, 4096], bf16)
            trash1 = wpool.tile([128, 4096], bf16)
            trash = [trash0, trash1]
            eps_sb = wpool.tile([128, 1], f32)
            nc.gpsimd.memset(eps_sb[:], EPS)
            invn_sb = wpool.tile([128, 1], f32)
            nc.gpsimd.memset(invn_sb[:], 1.0 / HW)
            zero_sb = wpool.tile([128, 1], f32)
            nc.gpsimd.memset(zero_sb[:], 0.0)

            st = {}  # per-chunk small tiles

            def emit_load(cc):
                xt = xpool.tile([128, PADF], bf16, tag="xt", name=f"xt{cc}")
                st[cc] = {"xt": xt}
                nc.gpsimd.memset(xt[:, 0:W], 0.0)
                nc.gpsimd.memset(xt[:, PADF - W:PADF], 0.0)
                for k in range(4):
                    nc.sync.dma_start(xt[:, W + k * 4096: W + (k + 1) * 4096],
                                      x_d[cc * 128:(cc + 1) * 128,
                                          k * 4096:(k + 1) * 4096])

            def emit_stats_block(cc, blk):
                # chunk-0 (prologue) path: bn_stats on DVE
                s = st[cc]
                if blk == 0:
                    s["stats6"] = spool.tile([128, 32 * 6], f32, tag="stats",
                                             name=f"st{cc}")
                xt = s["xt"]
                for j in range(8 * blk, 8 * blk + 8):
                    nc.vector.bn_stats(s["stats6"][:, j * 6:(j + 1) * 6],
                                       xt[:, W + j * 512: W + (j + 1) * 512])

            def emit_stats_act(cc, k):
                # steady-state path: ACT accumulates sum (Copy) and sumsq (Square)
                # of DMA slice k into acc cols; zero DVE cost.
                s = st[cc]
                if "acc" not in s:
                    s["acc"] = spool.tile([128, 8], f32, tag="acc", name=f"ac{cc}")
                xt = s["xt"]
                sl = xt[:, W + k * 4096: W + (k + 1) * 4096]
                nc.scalar.activation(trash[0][:], sl,
                                     mybir.ActivationFunctionType.Copy,
                                     accum_out=s["acc"][:, k:k + 1])
                nc.scalar.activation(trash[1][:], sl,
                                     mybir.ActivationFunctionType.Square,
                                     accum_out=s["acc"][:, k + 4:k + 5])

            def emit_chain_bn(cc):
                # prologue path: DVE is idle before the first conv, use it
                s = st[cc]
                mv = spool.tile([128, 2], f32, tag="mv", name=f"mv{cc}")
                nc.vector.bn_aggr(mv[:], s["stats6"][:].rearrange(
                    "p (h s) -> p h s", s=6))
                stdv = spool.tile([128, 1], f32, tag="stdv", name=f"sd{cc}")
                nc.scalar.activation(stdv[:], mv[:, 1:2],
                                     mybir.ActivationFunctionType.Sqrt,
                                     scale=float(HW) / float(HW - 1))
                stde = spool.tile([128, 1], f32, tag="stde", name=f"se{cc}")
                nc.vector.tensor_scalar_add(stde[:], stdv[:], EPS)
                rstd = spool.tile([128, 1], f32, tag="rstd", name=f"rs{cc}")
                nc.vector.reciprocal(rstd[:], stde[:])
                nmr = spool.tile([128, 1], f32, tag="nmr", name=f"nm{cc}")
                nc.vector.scalar_tensor_tensor(nmr[:], mv[:, 0:1], -1.0, rstd[:],
                                               op0=MULT, op1=MULT)
                s["rstd"] = rstd
                s["nmr"] = nmr

            def emit_chain_mix(cc):
                # prologue: merge DVE bn_stats (slices 0-1) with ACT accum
                # sums (slices 2-3); all combining on the (idle) DVE
                s = st[cc]
                acc = s["acc"]
                mv = spool.tile([128, 2], f32, tag="mv", name=f"mv{cc}")
                nc.vector.bn_aggr(mv[:], s["stats6"][:, 0:96].rearrange(
                    "p (h s) -> p h s", s=6))
                s1 = spool.tile([128, 2], f32, tag="s1", name=f"s1{cc}")
                nc.vector.tensor_add(s1[:, 0:1], acc[:, 2:3], acc[:, 3:4])
                nc.vector.tensor_add(s1[:, 1:2], acc[:, 6:7], acc[:, 7:8])
                mb = spool.tile([128, 1], f32, tag="mb", name=f"mb{cc}")
                nc.vector.tensor_scalar_mul(mb[:], s1[:, 0:1], 1.0 / HW)
                mean = spool.tile([128, 1], f32, tag="mean", name=f"me{cc}")
                nc.vector.scalar_tensor_tensor(mean[:], mv[:, 0:1], 0.5, mb[:],
                                               op0=MULT, op1=ADD)
                m2a = spool.tile([128, 1], f32, tag="m2a", name=f"ma{cc}")
                nc.vector.tensor_mul(m2a[:], mv[:, 0:1], mv[:, 0:1])
                e2a = spool.tile([128, 1], f32, tag="e2a", name=f"ea{cc}")
                nc.vector.tensor_scalar(e2a[:], mv[:, 1:2], m2a[:, 0:1], 0.5,
                                        op0=ADD, op1=MULT)
                e2b = spool.tile([128, 1], f32, tag="e2b", name=f"eb{cc}")
                nc.vector.tensor_scalar_mul(e2b[:], s1[:, 1:2], 1.0 / HW)
                ex2 = spool.tile([128, 1], f32, tag="ex2", name=f"ex{cc}")
                nc.vector.tensor_add(ex2[:], e2a[:], e2b[:])
                m2 = spool.tile([128, 1], f32, tag="m2", name=f"m2{cc}")
                nc.vector.tensor_mul(m2[:], mean[:], mean[:])
                var = spool.tile([128, 1], f32, tag="var", name=f"va{cc}")
                nc.vector.tensor_sub(var[:], ex2[:], m2[:])
                stdv = spool.tile([128, 1], f32, tag="stdv", name=f"sd{cc}")
                nc.scalar.activation(stdv[:], var[:],
                                     mybir.ActivationFunctionType.Sqrt,
                                     scale=float(HW) / float(HW - 1))
                stde = spool.tile([128, 1], f32, tag="stde", name=f"se{cc}")
                nc.vector.tensor_scalar_add(stde[:], stdv[:], EPS)
                rstd = spool.tile([128, 1], f32, tag="rstd", name=f"rs{cc}")
                nc.vector.reciprocal(rstd[:], stde[:])
                nmr = spool.tile([128, 1], f32, tag="nmr", name=f"nm{cc}")
                nc.vector.scalar_tensor_tensor(nmr[:], mean[:], -1.0, rstd[:],
                                               op0=MULT, op1=MULT)
                s["rstd"] = rstd
                s["nmr"] = nmr

            def emit_chain_acc(cc):
                # chain on GpSimd (+3 ACT transcendentals): DVE stays evac-only,
                # and the gp queue has nothing PE-critical to block
                s = st[cc]
                acc = s["acc"]
                g = nc.gpsimd
                t4 = spool.tile([128, 4], f32, tag="t4", name=f"t4{cc}")
                sm = spool.tile([128, 2], f32, tag="sm", name=f"sm{cc}")
                g.tensor_add(t4[:, 0:2], acc[:, 0:2], acc[:, 2:4])
                g.tensor_add(t4[:, 2:4], acc[:, 4:6], acc[:, 6:8])
                g.tensor_add(sm[:, 0:1], t4[:, 0:1], t4[:, 1:2])
                g.tensor_add(sm[:, 1:2], t4[:, 2:3], t4[:, 3:4])
                mean = spool.tile([128, 1], f32, tag="mean", name=f"me{cc}")
                g.tensor_mul(mean[:], sm[:, 0:1], invn_sb[:])
                ex2 = spool.tile([128, 1], f32, tag="ex2", name=f"ex{cc}")
                g.tensor_mul(ex2[:], sm[:, 1:2], invn_sb[:])
                m2 = spool.tile([128, 1], f32, tag="m2", name=f"m2{cc}")
                g.tensor_mul(m2[:], mean[:], mean[:])
                var = spool.tile([128, 1], f32, tag="var", name=f"va{cc}")
                g.tensor_sub(var[:], ex2[:], m2[:])
                stdv = spool.tile([128, 1], f32, tag="stdv", name=f"sd{cc}")
                nc.scalar.activation(stdv[:], var[:],
                                     mybir.ActivationFunctionType.Sqrt,
                                     scale=float(HW) / float(HW - 1))
                stde = spool.tile([128, 1], f32, tag="stde", name=f"se{cc}")
                g.tensor_add(stde[:], stdv[:], eps_sb[:])
                lg = spool.tile([128, 1], f32, tag="lg", name=f"lg{cc}")
                nc.scalar.activation(lg[:], stde[:],
                                     mybir.ActivationFunctionType.Ln)
                rstd = spool.tile([128, 1], f32, tag="rstd", name=f"rs{cc}")
                nc.scalar.activation(rstd[:], lg[:],
                                     mybir.ActivationFunctionType.Exp, scale=-1.0)
                nmrp = spool.tile([128, 1], f32, tag="nmrp", name=f"np{cc}")
                g.tensor_mul(nmrp[:], mean[:], rstd[:])
                nmr = spool.tile([128, 1], f32, tag="nmr", name=f"nm{cc}")
                g.tensor_sub(nmr[:], zero_sb[:], nmrp[:])
                s["rstd"] = rstd
                s["nmr"] = nmr

            def emit_norm(cc, engines=(None,)):
                # 8 row-band pieces (conv is gated on the WHOLE normalize anyway
                # -- Tile tracks xt coarsely -- so minimize serial latency)
                s = st[cc]
                full = s["xt"][:, W:W + HW].rearrange("p (b r w) -> p b r w",
                                                      b=4, r=32)
                for q in range(8):
                    eng = engines[q % len(engines)]
                    tgt = full[:, :, 4 * q:4 * q + 4, :]
                    eng.tensor_scalar(tgt, tgt, s["rstd"][:, 0:1],
                                      s["nmr"][:, 0:1], op0=MULT, op1=ADD)

            def emit_span_mms(cc, q):
                # span q: four 64x64 array tiles = 2 channel sub-chunk PAIRS
                # (row groups R, 16 groups block-diag each) x 2 spatial halves
                # (col groups C); C covers spatial tile 16C + q
                xt = st[cc]["xt"]
                pb = [pspool.tile([128, 512], f32, tag="pb",
                                  name=f"pb{cc}_{q}_{R}") for R in range(2)]
                for ti, (dh, dwi) in enumerate(TAPS):
                    start, stop = (ti == 0), (ti == NTAP - 1)
                    tapi = dh * 3 + dwi
                    for R in range(2):
                        lhsT = w_sb[64 * R:64 * R + 64,
                                    (cc * NTAP + tapi) * 64:
                                    (cc * NTAP + tapi) * 64 + 64]
                        for Cg in range(2):
                            t = 16 * Cg + q
                            base = (4 * t + dh) * W
                            outp = pb[R][64 * Cg:64 * Cg + 64, :]
                            tp = (64 * R, 64 * Cg)
                            if dwi == 1:
                                nc.tensor.matmul(
                                    outp, lhsT,
                                    xt[64 * R:64 * R + 64, base:base + 512],
                                    start=start, stop=stop, tile_position=tp)
                            else:
                                o3 = outp.rearrange("p (h w) -> p h w", w=W)
                                r3 = xt[64 * R:64 * R + 64,
                                        base:base + 512].rearrange(
                                            "p (h w) -> p h w", w=W)
                                if dwi == 0:   # dw=-1
                                    nc.tensor.matmul(
                                        o3[:, :, 1:W], lhsT, r3[:, :, 0:W - 1],
                                        start=start, stop=stop,
                                        skip_group_check=True, tile_position=tp)
                                else:          # dw=+1
                                    nc.tensor.matmul(
                                        o3[:, :, 0:W - 1], lhsT, r3[:, :, 1:W],
                                        start=start, stop=stop,
                                        skip_group_check=True, tile_position=tp)
                return pb

            def emit_evac(cc, q, pb, om):
                for R in range(2):
                    bias_ap = bias_sb[:, cc * 2 + R: cc * 2 + R + 1]
                    dst = om[:, R * 8192 + q * 512: R * 8192 + q * 512 + 512]
                    nc.vector.tensor_scalar_add(dst, pb[R][:, :], bias_ap)

            def emit_out(cc, om, hh):
                for Cg in range(2):
                    nc.sync.dma_start(
                        out_v[cc, Cg, hh],
                        om[64 * Cg:64 * Cg + 64, :].rearrange(
                            "p (R hh e) -> p R hh e", hh=2, e=4096)[:, :, hh, :])

            # ---- prologue: chunk 0 (and chunk 1 load) up front
            # prologue: chunk 0 stats on DVE (idle), chunk 1 stats on ACT, both
            # chains done before conv(0) ends; chunk-0 norm split gp/DVE
            emit_load(0)
            emit_load(1)
            for blk in range(4):
                emit_stats_block(0, blk)
            emit_chain_bn(0)
            emit_norm(0, engines=(nc.gpsimd, nc.vector))
            for k in range(4):
                emit_stats_act(1, k)

            # steady: loads + ACT stats run two chunks ahead; chain(n) at
            # (n-1, q1) when its inputs are already done; norm(n) at (n-1, q3)
            for cc in range(NCH):
                om = opool.tile([128, 4 * 4096], bf16, tag="om", name=f"om{cc}")
                for q in range(16):
                    pb = emit_span_mms(cc, q)
                    emit_evac(cc, q, pb, om)
                    if q == 0 and cc + 2 < NCH:
                        emit_load(cc + 2)
                    if q == 1 and cc + 1 < NCH:
                        emit_chain_acc(cc + 1)
                    if q in (6, 8, 10, 12) and cc + 2 < NCH:
                        emit_stats_act(cc + 2, q // 2 - 3)
                    if q == 6 and cc + 1 < NCH:
                        emit_norm(cc + 1, engines=(nc.gpsimd, nc.vector))
                    if q == 7:
                        emit_out(cc, om, 0)
                emit_out(cc, om, 1)
    nc.compile()
    return nc


def _pack_inputs(x, dw, pw, biases):
    """Host-side: fuse pw o dw, scatter into block-diag 32x32 lhsT tiles."""
    G = 128
    dwr = dw.reshape(B, G, 4, 4, 3, 3)          # [b, g, m, i, kh, kw]
    pwr = pw.reshape(B, G, 4, 4)                # [b, g, j, m]
    eff = np.einsum('bgjm,bgmikl->bgjikl', pwr, dwr)  # [b, g, j, i, kh, kw]
    # 64x64 block-diag tiles: w_host[b, 64R + 4gl + i, (cc*9+tap)*64 + 4gl + j]
    w_host = np.zeros((B, 128, NCH * NTAP * 64), dtype=np.float32)
    wv = w_host.reshape(B, 2, 16, 4, NCH, NTAP, 16, 4)  # [b,R,gl_k,i,cc,tap,gl_m,j]
    er = eff.reshape(B, NCH, 2, 16, 4, 4, NTAP)         # [b, cc, R, gl, j, i, tap]
    for gl in range(16):
        e = er[:, :, :, gl]                     # [b, cc, R, j, i, tap]
        wv[:, :, gl, :, :, :, gl, :] = e.transpose(0, 2, 4, 1, 5, 3)
    bias_host = np.zeros((B, 128, 8), dtype=np.float32)
    bfull = biases.reshape(B, C)
    p = np.arange(128)
    for cc in range(NCH):
        for R in range(2):
            bias_host[:, :, cc * 2 + R] = bfull[:, cc * 128 + 64 * R + (p % 64)]
    return w_host.astype(ml_dtypes.bfloat16), bias_host


def kernel(x, dw_kernels, pw_kernels, biases):
    from concourse.bass_utils import run_bass_kernel_spmd

    x = np.ascontiguousarray(np.asarray(x, dtype=np.float32))
    dw = np.asarray(dw_kernels, dtype=np.float32)
    pw = np.asarray(pw_kernels, dtype=np.float32)
    bs = np.asarray(biases, dtype=np.float32)

    if "nc" not in _CACHE:
        _CACHE["nc"] = _build_program()
    nc = _CACHE["nc"]

    w_host, bias_host = _pack_inputs(x, dw, pw, bs)
    xb = x.reshape(B, C, HW).astype(ml_dtypes.bfloat16)
    in_maps = [{"x": xb[i],
                "w": w_host[i],
                "bias": bias_host[i]} for i in range(B)]
    res = run_bass_kernel_spmd(nc, in_maps, core_ids=list(range(B)),
                               trace=bool(int(os.environ.get("KTRACE", "0"))))
    _CACHE["last_result"] = res
    out = np.stack([res.results[i]["out"].astype(np.float32).reshape(C, H, W)
                    for i in range(B)])
    return out


# revision 72
# speedup vs baseline: 1.3113x; 1.3113x over previous
"""AdaConv2D (instance-norm -> grouped 3x3 conv -> grouped 1x1 conv -> bias) on 8 TRN2 cores.

Strategy (pure data parallel: batch dim sharded, 1 sample per NeuronCore, no
collectives; measured ~237-245us HW exec, rel err ~2.2e-3 vs the f32 reference):

  Host-side prep (inside kernel(), before launch):
  - Fuse the grouped 1x1 conv into the grouped 3x3 conv: both share the same
    4-channel group partition, so eff[g,j,i,kh,kw] = sum_m pw[g,j,m]*dw[g,m,i,kh,kw]
    gives ONE effective grouped 3x3 conv.
  - Pack eff into block-diagonal 32x32 bf16 lhsT tiles (8 groups of 4x4 per tile),
    one per (channel sub-chunk r, tap); x is converted to bf16 (rel-err budget
    2e-2 >> bf16 rounding; halves input DMA traffic).

  Device, per 128-channel chunk (4 chunks/sample), software-pipelined 3 deep with
  STRICT engine specialization (each engine executes its scheduled queue in order,
  so cross-chunk work must never queue behind blocking predecessors):
  - Sync (HWDGE): input DMA, 4 slices/chunk, issued two chunks ahead; output DMA.
  - ACT: instance-norm statistics via Copy/Square activations with accum_out
    (sum and sum-of-squares per slice, zero DVE cost), issued two chunks ahead.
  - GpSimd: halo memsets, the mean/var/rstd chain (tensor-tensor ops + 3 ACT
    transcendentals: rstd = exp(-ln(sqrt(var*N/(N-1))+eps))), and the normalize
    pass (8 pieces, tensor_scalar x*rstd + (-mean*rstd)), one chunk ahead.
  - DVE: PSUM eviction ONLY (psum + per-partition bias -> bf16 staging tile);
    anything else on this queue stalls TensorE via PSUM back-pressure.
  - TensorE: conv as 4 concurrent 64x64 tile_position matmuls per span =
    2 channel sub-chunk PAIRS (row groups; contraction = 64 channels,
    16 groups block-diag) x 2 spatial halves (col groups).  9 taps = shifted
    APs on a row-padded SBUF layout (1 zero halo row above/below, rows of 128
    contiguous), accumulated in PSUM (start on the first dw=0 tap).  W-edge
    padding is done by SHRINKING the free dim of dw=+-1 taps (edge output
    columns simply don't receive those taps).  Spatial tile of col group C at
    span q is t = 16C + q, so each partition half owns a contiguous spatial
    half, giving 8KB-contiguous output DMA runs.  The PE is instruction-issue
    bound, so fewer/bigger tiles beat 16x 32x32 (measured 238us -> 204us);
    LDWEIGHTS is 1:1 with matmuls (walrus ldw-opt crashes, N=1024 moving
    operand is ISA-rejected for the one-PSUM-bank limit).
  - Output staged in bf16 (halves output traffic), stored per quarter-drain;
    host upcasts to f32.
"""
import os
import sys
import numpy as np
import ml_dtypes

if "/opt/trn_rl_repo" not in sys.path:
    sys.path.insert(0, "/opt/trn_rl_repo")

B, C, H, W = 8, 512, 128, 128
HW = H * W            # 16384
NCH = 4               # 128-channel chunks per sample
NTAP = 9
ROWS_PAD = H + 2      # 130 rows of 128 in padded SBUF layout
PADF = ROWS_PAD * W   # 16640 elems per partition
EPS = 1e-7
# taps ordered so the first three are dw=0 (full-width writes -> correct PSUM init)
TAPS = [(0, 1), (1, 1), (2, 1), (0, 0), (1, 0), (2, 0), (0, 2), (1, 2), (2, 2)]

_CACHE = {}


def _build_program():
    import concourse.bass as bass
    import concourse.tile as tile
    from concourse import bacc, mybir

    f32 = mybir.dt.float32
    bf16 = mybir.dt.bfloat16
    MULT = mybir.AluOpType.mult
    ADD = mybir.AluOpType.add
    SUB = mybir.AluOpType.subtract
    IDENT = mybir.ActivationFunctionType.Identity
    nc = bacc.Bacc("TRN2", target_bir_lowering=False, debug=False,
                   enable_asserts=False, num_devices=8)

    x_d = nc.dram_tensor("x", [C, HW], bf16, kind="ExternalInput")
    w_d = nc.dram_tensor("w", [128, NCH * NTAP * 64], bf16, kind="ExternalInput")
    b_d = nc.dram_tensor("bias", [128, 8], f32, kind="ExternalInput")
    out_d = nc.dram_tensor("out", [C, HW], bf16, kind="ExternalOutput")

    # store view: [cc, Ch(spatial half), hh(drain half), p, R, e(4096)]
    out_v = out_d[:].rearrange("(a R p) (Ch hh e) -> a Ch hh p R e", a=NCH, R=2,
                               p=64, Ch=2, hh=2, e=4096)

    with tile.TileContext(nc) as tc:
        with (
            tc.tile_pool(name="xpool", bufs=3) as xpool,
            tc.tile_pool(name="wpool", bufs=1) as wpool,
            tc.tile_pool(name="spool", bufs=3) as spool,
            tc.tile_pool(name="opool", bufs=2) as opool,
            tc.tile_pool(name="psum", bufs=8, space=bass.MemorySpace.PSUM) as pspool,
        ):
            w_sb = wpool.tile([128, NCH * NTAP * 64], bf16)
            nc.sync.dma_start(w_sb[:], w_d[:])
            bias_sb = wpool.tile([128, 8], f32)
            nc.sync.dma_start(bias_sb[:], b_d[:])
            trash0 = wpool.tile([128, 4096], bf16)
            trash1 = wpool.tile([128, 4096], bf16)
            trash = [trash0, trash1]
            eps_sb = wpool.tile([128, 1], f32)
            nc.gpsimd.memset(eps_sb[:], EPS)
            invn_sb = wpool.tile([128, 1], f32)
            nc.gpsimd.memset(invn_sb[:], 1.0 / HW)
            zero_sb = wpool.tile([128, 1], f32)
            nc.gpsimd.memset(zero_sb[:], 0.0)

            st = {}  # per-chunk small tiles

            def emit_load(cc):
                xt = xpool.tile([128, PADF], bf16, tag="xt", name=f"xt{cc}")
                st[cc] = {"xt": xt}
                nc.gpsimd.memset(xt[:, 0:W], 0.0)
                nc.gpsimd.memset(xt[:, PADF - W:PADF], 0.0)
                for k in range(4):
                    nc.sync.dma_start(xt[:, W + k * 4096: W + (k + 1) * 4096],
                                      x_d[cc * 128:(cc + 1) * 128,
                                          k * 4096:(k + 1) * 4096])

            def emit_stats_block(cc, blk):
                # chunk-0 (prologue) path: bn_stats on DVE
                s = st[cc]
                if blk == 0:
                    s["stats6"] = spool.tile([128, 32 * 6], f32, tag="stats",
                                             name=f"st{cc}")
                xt = s["xt"]
                for j in range(8 * blk, 8 * blk + 8):
                    nc.vector.bn_stats(s["stats6"][:, j * 6:(j + 1) * 6],
                                       xt[:, W + j * 512: W + (j + 1) * 512])

            def emit_stats_act(cc, k):
                # steady-state path: ACT accumulates sum (Copy) and sumsq (Square)
                # of DMA slice k into acc cols; zero DVE cost.
                s = st[cc]
                if "acc" not in s:
                    s["acc"] = spool.tile([128, 8], f32, tag="acc", name=f"ac{cc}")
                xt = s["xt"]
                sl = xt[:, W + k * 4096: W + (k + 1) * 4096]
                nc.scalar.activation(trash[0][:], sl,
                                     mybir.ActivationFunctionType.Copy,
                                     accum_out=s["acc"][:, k:k + 1])
                nc.scalar.activation(trash[1][:], sl,
                                     mybir.ActivationFunctionType.Square,
                                     accum_out=s["acc"][:, k + 4:k + 5])

            def emit_chain_bn(cc):
                # prologue path: DVE is idle before the first conv, use it
                s = st[cc]
                mv = spool.tile([128, 2], f32, tag="mv", name=f"mv{cc}")
                nc.vector.bn_aggr(mv[:], s["stats6"][:].rearrange(
                    "p (h s) -> p h s", s=6))
                stdv = spool.tile([128, 1], f32, tag="stdv", name=f"sd{cc}")
                nc.scalar.activation(stdv[:], mv[:, 1:2],
                                     mybir.ActivationFunctionType.Sqrt,
                                     scale=float(HW) / float(HW - 1))
                stde = spool.tile([128, 1], f32, tag="stde", name=f"se{cc}")
                nc.vector.tensor_scalar_add(stde[:], stdv[:], EPS)
                rstd = spool.tile([128, 1], f32, tag="rstd", name=f"rs{cc}")
                nc.vector.reciprocal(rstd[:], stde[:])
                nmr = spool.tile([128, 1], f32, tag="nmr", name=f"nm{cc}")
                nc.vector.scalar_tensor_tensor(nmr[:], mv[:, 0:1], -1.0, rstd[:],
                                               op0=MULT, op1=MULT)
                s["rstd"] = rstd
                s["nmr"] = nmr

            def emit_chain_mix(cc):
                # prologue: merge DVE bn_stats (slices 0-1) with ACT accum
                # sums (slices 2-3); all combining on the (idle) DVE
                s = st[cc]
                acc = s["acc"]
                mv = spool.tile([128, 2], f32, tag="mv", name=f"mv{cc}")
                nc.vector.bn_aggr(mv[:], s["stats6"][:, 0:96].rearrange(
                    "p (h s) -> p h s", s=6))
                s1 = spool.tile([128, 2], f32, tag="s1", name=f"s1{cc}")
                nc.vector.tensor_add(s1[:, 0:1], acc[:, 2:3], acc[:, 3:4])
                nc.vector.tensor_add(s1[:, 1:2], acc[:, 6:7], acc[:, 7:8])
                mb = spool.tile([128, 1], f32, tag="mb", name=f"mb{cc}")
                nc.vector.tensor_scalar_mul(mb[:], s1[:, 0:1], 1.0 / HW)
                mean = spool.tile([128, 1], f32, tag="mean", name=f"me{cc}")
                nc.vector.scalar_tensor_tensor(mean[:], mv[:, 0:1], 0.5, mb[:],
                                               op0=MULT, op1=ADD)
                m2a = spool.tile([128, 1], f32, tag="m2a", name=f"ma{cc}")
                nc.vector.tensor_mul(m2a[:], mv[:, 0:1], mv[:, 0:1])
                e2a = spool.tile([128, 1], f32, tag="e2a", name=f"ea{cc}")
                nc.vector.tensor_scalar(e2a[:], mv[:, 1:2], m2a[:, 0:1], 0.5,
                                        op0=ADD, op1=MULT)
                e2b = spool.tile([128, 1], f32, tag="e2b", name=f"eb{cc}")
                nc.vector.tensor_scalar_mul(e2b[:], s1[:, 1:2], 1.0 / HW)
                ex2 = spool.tile([128, 1], f32, tag="ex2", name=f"ex{cc}")
                nc.vector.tensor_add(ex2[:], e2a[:], e2b[:])
                m2 = spool.tile([128, 1], f32, tag="m2", name=f"m2{cc}")
                nc.vector.tensor_mul(m2[:], mean[:], mean[:])
                var = spool.tile([128, 1], f32, tag="var", name=f"va{cc}")
                nc.vector.tensor_sub(var[:], ex2[:], m2[:])
                stdv = spool.tile([128, 1], f32, tag="stdv", name=f"sd{cc}")
                nc.scalar.activation(stdv[:], var[:],
                                     mybir.ActivationFunctionType.Sqrt,
                                     scale=float(HW) / float(HW - 1))
                stde = spool.tile([128, 1], f32, tag="stde", name=f"se{cc}")
                nc.vector.tensor_scalar_add(stde[:], stdv[:], EPS)
                rstd = spool.tile([128, 1], f32, tag="rstd", name=f"rs{cc}")
                nc.vector.reciprocal(rstd[:], stde[:])
                nmr = spool.tile([128, 1], f32, tag="nmr", name=f"nm{cc}")
                nc.vector.scalar_tensor_tensor(nmr[:], mean[:], -1.0, rstd[:],
                                               op0=MULT, op1=MULT)
                s["rstd"] = rstd
                s["nmr"] = nmr

            def emit_chain_acc(cc):
                # chain on GpSimd (+3 ACT transcendentals): DVE stays evac-only,
                # and the gp queue has nothing PE-critical to block
                s = st[cc]
                acc = s["acc"]
                g = nc.gpsimd
                t4 = spool.tile([128, 4], f32, tag="t4", name=f"t4{cc}")
                sm = spool.tile([128, 2], f32, tag="sm", name=f"sm{cc}")
                g.tensor_add(t4[:, 0:2], acc[:, 0:2], acc[:, 2:4])
                g.tensor_add(t4[:, 2:4], acc[:, 4:6], acc[:, 6:8])
                g.tensor_add(sm[:, 0:1], t4[:, 0:1], t4[:, 1:2])
                g.tensor_add(sm[:, 1:2], t4[:, 2:3], t4[:, 3:4])
                mean = spool.tile([128, 1], f32, tag="mean", name=f"me{cc}")
                g.tensor_mul(mean[:], sm[:, 0:1], invn_sb[:])
                ex2 = spool.tile([128, 1], f32, tag="ex2", name=f"ex{cc}")
                g.tensor_mul(ex2[:], sm[:, 1:2], invn_sb[:])
                m2 = spool.tile([128, 1], f32, tag="m2", name=f"m2{cc}")
                g.tensor_mul(m2[:], mean[:], mean[:])
                var = spool.tile([128, 1], f32, tag="var", name=f"va{cc}")
                g.tensor_sub(var[:], ex2[:], m2[:])
                stdv = spool.tile([128, 1], f32, tag="stdv", name=f"sd{cc}")
                nc.scalar.activation(stdv[:], var[:],
                                     mybir.ActivationFunctionType.Sqrt,
                                     scale=float(HW) / float(HW - 1))
                stde = spool.tile([128, 1], f32, tag="stde", name=f"se{cc}")
                g.tensor_add(stde[:], stdv[:], eps_sb[:])
                lg = spool.tile([128, 1], f32, tag="lg", name=f"lg{cc}")
                nc.scalar.activation(lg[:], stde[:],
                                     mybir.ActivationFunctionType.Ln)
                rstd = spool.tile([128, 1], f32, tag="rstd", name=f"rs{cc}")
                nc.scalar.activation(rstd[:], lg[:],
                                     mybir.ActivationFunctionType.Exp, scale=-1.0)
                nmrp = spool.tile([128, 1], f32, tag="nmrp", name=f"np{cc}")
                g.tensor_mul(nmrp[:], mean[:], rstd[:])
                nmr = spool.tile([128, 1], f32, tag="nmr", name=f"nm{cc}")
                g.tensor_sub(nmr[:], zero_sb[:], nmrp[:])
                s["rstd"] = rstd
                s["nmr"] = nmr

            def emit_norm(cc, engines=(None,)):
                # 8 row-band pieces (conv is gated on the WHOLE normalize anyway
                # -- Tile tracks xt coarsely -- so minimize serial latency)
                s = st[cc]
                full = s["xt"][:, W:W + HW].rearrange("p (b r w) -> p b r w",
                                                      b=4, r=32)
                for q in range(8):
                    eng = engines[q % len(engines)]
                    tgt = full[:, :, 4 * q:4 * q + 4, :]
                    eng.tensor_scalar(tgt, tgt, s["rstd"][:, 0:1],
                                      s["nmr"][:, 0:1], op0=MULT, op1=ADD)

            def emit_span_mms(cc, q):
                # span q: four 64x64 array tiles = 2 channel sub-chunk PAIRS
                # (row groups R, 16 groups block-diag each) x 2 spatial halves
                # (col groups C); C covers spatial tile 16C + q
                xt = st[cc]["xt"]
                pb = [pspool.tile([128, 512], f32, tag="pb",
                                  name=f"pb{cc}_{q}_{R}") for R in range(2)]
                for ti, (dh, dwi) in enumerate(TAPS):
                    start, stop = (ti == 0), (ti == NTAP - 1)
                    tapi = dh * 3 + dwi
                    for R in range(2):
                        lhsT = w_sb[64 * R:64 * R + 64,
                                    (cc * NTAP + tapi) * 64:
                                    (cc * NTAP + tapi) * 64 + 64]
                        for Cg in range(2):
                            t = 16 * Cg + q
                            base = (4 * t + dh) * W
                            outp = pb[R][64 * Cg:64 * Cg + 64, :]
                            tp = (64 * R, 64 * Cg)
                            if dwi == 1:
                                nc.tensor.matmul(
                                    outp, lhsT,
                                    xt[64 * R:64 * R + 64, base:base + 512],
                                    start=start, stop=stop, tile_position=tp)
                            else:
                                o3 = outp.rearrange("p (h w) -> p h w", w=W)
                                r3 = xt[64 * R:64 * R + 64,
                                        base:base + 512].rearrange(
                                            "p (h w) -> p h w", w=W)
                                if dwi == 0:   # dw=-1
                                    nc.tensor.matmul(
                                        o3[:, :, 1:W], lhsT, r3[:, :, 0:W - 1],
                                        start=start, stop=stop,
                                        skip_group_check=True, tile_position=tp)
                                else:          # dw=+1
                                    nc.tensor.matmul(
                                        o3[:, :, 0:W - 1], lhsT, r3[:, :, 1:W],
                                        start=start, stop=stop,
                                        skip_group_check=True, tile_position=tp)
                return pb

            def emit_evac(cc, q, pb, om):
                for R in range(2):
                    bias_ap = bias_sb[:, cc * 2 + R: cc * 2 + R + 1]
                    dst = om[:, R * 8192 + q * 512: R * 8192 + q * 512 + 512]
                    nc.vector.tensor_scalar_add(dst, pb[R][:, :], bias_ap)

            def emit_out(cc, om, hh):
                for Cg in range(2):
                    nc.sync.dma_start(
                        out_v[cc, Cg, hh],
                        om[64 * Cg:64 * Cg + 64, :].rearrange(
                            "p (R hh e) -> p R hh e", hh=2, e=4096)[:, :, hh, :])

            # ---- prologue: chunk 0 (and chunk 1 load) up front
            # prologue: chunk 0 stats on DVE (idle), chunk 1 stats on ACT, both
            # chains done before conv(0) ends; chunk-0 norm split gp/DVE
            emit_load(0)
            emit_load(1)
            for blk in range(4):
                emit_stats_block(0, blk)
            emit_chain_bn(0)
            emit_norm(0, engines=(nc.gpsimd, nc.vector))
            for k in range(4):
                emit_stats_act(1, k)

            # steady: loads + ACT stats run two chunks ahead; chain(n) at
            # (n-1, q1) when its inputs are already done; norm(n) at (n-1, q3)
            for cc in range(NCH):
                om = opool.tile([128, 4 * 4096], bf16, tag="om", name=f"om{cc}")
                for q in range(16):
                    pb = emit_span_mms(cc, q)
                    emit_evac(cc, q, pb, om)
                    if q == 0 and cc + 2 < NCH:
                        emit_load(cc + 2)
                    if q == 1 and cc + 1 < NCH:
                        emit_chain_acc(cc + 1)
                    if q in (2, 4, 6, 8) and cc + 2 < NCH:
                        emit_stats_act(cc + 2, q // 2 - 1)
                    if q == 6 and cc + 1 < NCH:
                        emit_norm(cc + 1, engines=(nc.gpsimd,))
                    if q == 7:
                        emit_out(cc, om, 0)
                emit_out(cc, om, 1)
    nc.compile()
    return nc


def _pack_inputs(x, dw, pw, biases):
    """Host-side: fuse pw o dw, scatter into block-diag 32x32 lhsT tiles."""
    G = 128
    dwr = dw.reshape(B, G, 4, 4, 3, 3)          # [b, g, m, i, kh, kw]
    pwr = pw.reshape(B, G, 4, 4)                # [b, g, j, m]
    eff = np.einsum('bgjm,bgmikl->bgjikl', pwr, dwr)  # [b, g, j, i, kh, kw]
    # 64x64 block-diag tiles: w_host[b, 64R + 4gl + i, (cc*9+tap)*64 + 4gl + j]
    w_host = np.zeros((B, 128, NCH * NTAP * 64), dtype=np.float32)
    wv = w_host.reshape(B, 2, 16, 4, NCH, NTAP, 16, 4)  # [b,R,gl_k,i,cc,tap,gl_m,j]
    er = eff.reshape(B, NCH, 2, 16, 4, 4, NTAP)         # [b, cc, R, gl, j, i, tap]
    for gl in range(16):
        e = er[:, :, :, gl]                     # [b, cc, R, j, i, tap]
        wv[:, :, gl, :, :, :, gl, :] = e.transpose(0, 2, 4, 1, 5, 3)
    bias_host = np.zeros((B, 128, 8), dtype=np.float32)
    bfull = biases.reshape(B, C)
    p = np.arange(128)
    for cc in range(NCH):
        for R in range(2):
            bias_host[:, :, cc * 2 + R] = bfull[:, cc * 128 + 64 * R + (p % 64)]
    return w_host.astype(ml_dtypes.bfloat16), bias_host


def kernel(x, dw_kernels, pw_kernels, biases):
    from concourse.bass_utils import run_bass_kernel_spmd

    x = np.ascontiguousarray(np.asarray(x, dtype=np.float32))
    dw = np.asarray(dw_kernels, dtype=np.float32)
    pw = np.asarray(pw_kernels, dtype=np.float32)
    bs = np.asarray(biases, dtype=np.float32)

    if "nc" not in _CACHE:
        _CACHE["nc"] = _build_program()
    nc = _CACHE["nc"]

    w_host, bias_host = _pack_inputs(x, dw, pw, bs)
    xb = x.reshape(B, C, HW).astype(ml_dtypes.bfloat16)
    in_maps = [{"x": xb[i],
                "w": w_host[i],
                "bias": bias_host[i]} for i in range(B)]
    res = run_bass_kernel_spmd(nc, in_maps, core_ids=list(range(B)),
                               trace=bool(int(os.environ.get("KTRACE", "0"))))
    _CACHE["last_result"] = res
    out = np.stack([res.results[i]["out"].astype(np.float32).reshape(C, H, W)
                    for i in range(B)])
    return out


# revision 73
# speedup vs baseline: 1.3386x; 1.0208x over previous
"""AdaConv2D (instance-norm -> grouped 3x3 conv -> grouped 1x1 conv -> bias) on 8 TRN2 cores.

Strategy (pure data parallel: batch dim sharded, 1 sample per NeuronCore, no
collectives; measured ~237-245us HW exec, rel err ~2.2e-3 vs the f32 reference):

  Host-side prep (inside kernel(), before launch):
  - Fuse the grouped 1x1 conv into the grouped 3x3 conv: both share the same
    4-channel group partition, so eff[g,j,i,kh,kw] = sum_m pw[g,j,m]*dw[g,m,i,kh,kw]
    gives ONE effective grouped 3x3 conv.
  - Pack eff into block-diagonal 32x32 bf16 lhsT tiles (8 groups of 4x4 per tile),
    one per (channel sub-chunk r, tap); x is converted to bf16 (rel-err budget
    2e-2 >> bf16 rounding; halves input DMA traffic).

  Device, per 128-channel chunk (4 chunks/sample), software-pipelined 3 deep with
  STRICT engine specialization (each engine executes its scheduled queue in order,
  so cross-chunk work must never queue behind blocking predecessors):
  - Sync (HWDGE): input DMA, 4 slices/chunk, issued two chunks ahead; output DMA.
  - ACT: instance-norm statistics via Copy/Square activations with accum_out
    (sum and sum-of-squares per slice, zero DVE cost), issued two chunks ahead.
  - GpSimd: halo memsets, the mean/var/rstd chain (tensor-tensor ops + 3 ACT
    transcendentals: rstd = exp(-ln(sqrt(var*N/(N-1))+eps))), and the normalize
    pass (8 pieces, tensor_scalar x*rstd + (-mean*rstd)), one chunk ahead.
  - DVE: PSUM eviction ONLY (psum + per-partition bias -> bf16 staging tile);
    anything else on this queue stalls TensorE via PSUM back-pressure.
  - TensorE: conv as 4 concurrent 64x64 tile_position matmuls per span =
    2 channel sub-chunk PAIRS (row groups; contraction = 64 channels,
    16 groups block-diag) x 2 spatial halves (col groups).  9 taps = shifted
    APs on a row-padded SBUF layout (1 zero halo row above/below, rows of 128
    contiguous), accumulated in PSUM (start on the first dw=0 tap).  W-edge
    padding is done by SHRINKING the free dim of dw=+-1 taps (edge output
    columns simply don't receive those taps).  Spatial tile of col group C at
    span q is t = 16C + q, so each partition half owns a contiguous spatial
    half, giving 8KB-contiguous output DMA runs.  The PE is instruction-issue
    bound, so fewer/bigger tiles beat 16x 32x32 (measured 238us -> 204us);
    LDWEIGHTS is 1:1 with matmuls (walrus ldw-opt crashes, N=1024 moving
    operand is ISA-rejected for the one-PSUM-bank limit).
  - Output staged in bf16 (halves output traffic), stored per quarter-drain;
    host upcasts to f32.
"""
import os
import sys
import numpy as np
import ml_dtypes

if "/opt/trn_rl_repo" not in sys.path:
    sys.path.insert(0, "/opt/trn_rl_repo")

B, C, H, W = 8, 512, 128, 128
HW = H * W            # 16384
NCH = 4               # 128-channel chunks per sample
NTAP = 9
ROWS_PAD = H + 2      # 130 rows of 128 in padded SBUF layout
PADF = ROWS_PAD * W   # 16640 elems per partition
EPS = 1e-7
# taps ordered so the first three are dw=0 (full-width writes -> correct PSUM init)
TAPS = [(0, 1), (1, 1), (2, 1), (0, 0), (1, 0), (2, 0), (0, 2), (1, 2), (2, 2)]

_CACHE = {}


def _build_program():
    import concourse.bass as bass
    import concourse.tile as tile
    from concourse import bacc, mybir

    f32 = mybir.dt.float32
    bf16 = mybir.dt.bfloat16
    MULT = mybir.AluOpType.mult
    ADD = mybir.AluOpType.add
    SUB = mybir.AluOpType.subtract
    IDENT = mybir.ActivationFunctionType.Identity
    nc = bacc.Bacc("TRN2", target_bir_lowering=False, debug=False,
                   enable_asserts=False, num_devices=8)

    x_d = nc.dram_tensor("x", [C, HW], bf16, kind="ExternalInput")
    w_d = nc.dram_tensor("w", [128, NCH * NTAP * 64], bf16, kind="ExternalInput")
    b_d = nc.dram_tensor("bias", [128, 8], f32, kind="ExternalInput")
    out_d = nc.dram_tensor("out", [C, HW], bf16, kind="ExternalOutput")

    # store view: [cc, Ch(spatial half), hh(drain half), p, R, e(4096)]
    out_v = out_d[:].rearrange("(a R p) (Ch hh e) -> a Ch hh p R e", a=NCH, R=2,
                               p=64, Ch=2, hh=2, e=4096)

    with tile.TileContext(nc) as tc:
        with (
            tc.tile_pool(name="xpool", bufs=3) as xpool,
            tc.tile_pool(name="wpool", bufs=1) as wpool,
            tc.tile_pool(name="spool", bufs=3) as spool,
            tc.tile_pool(name="opool", bufs=2) as opool,
            tc.tile_pool(name="psum", bufs=8, space=bass.MemorySpace.PSUM) as pspool,
        ):
            w_sb = wpool.tile([128, NCH * NTAP * 64], bf16)
            nc.sync.dma_start(w_sb[:], w_d[:])
            bias_sb = wpool.tile([128, 8], f32)
            nc.sync.dma_start(bias_sb[:], b_d[:])
            trash0 = wpool.tile([128, 4096], bf16)
            trash1 = wpool.tile([128, 4096], bf16)
            trash = [trash0, trash1]
            eps_sb = wpool.tile([128, 1], f32)
            nc.gpsimd.memset(eps_sb[:], EPS)
            invn_sb = wpool.tile([128, 1], f32)
            nc.gpsimd.memset(invn_sb[:], 1.0 / HW)
            zero_sb = wpool.tile([128, 1], f32)
            nc.gpsimd.memset(zero_sb[:], 0.0)

            st = {}  # per-chunk small tiles

            def emit_load(cc):
                xt = xpool.tile([128, PADF], bf16, tag="xt", name=f"xt{cc}")
                st[cc] = {"xt": xt}
                nc.gpsimd.memset(xt[:, 0:W], 0.0)
                nc.gpsimd.memset(xt[:, PADF - W:PADF], 0.0)
                for k in range(4):
                    nc.sync.dma_start(xt[:, W + k * 4096: W + (k + 1) * 4096],
                                      x_d[cc * 128:(cc + 1) * 128,
                                          k * 4096:(k + 1) * 4096])

            def emit_stats_block(cc, blk):
                # chunk-0 (prologue) path: bn_stats on DVE
                s = st[cc]
                if blk == 0:
                    s["stats6"] = spool.tile([128, 32 * 6], f32, tag="stats",
                                             name=f"st{cc}")
                xt = s["xt"]
                for j in range(8 * blk, 8 * blk + 8):
                    nc.vector.bn_stats(s["stats6"][:, j * 6:(j + 1) * 6],
                                       xt[:, W + j * 512: W + (j + 1) * 512])

            def emit_stats_act(cc, k):
                # steady-state path: ACT accumulates sum (Copy) and sumsq (Square)
                # of DMA slice k into acc cols; zero DVE cost.
                s = st[cc]
                if "acc" not in s:
                    s["acc"] = spool.tile([128, 8], f32, tag="acc", name=f"ac{cc}")
                xt = s["xt"]
                sl = xt[:, W + k * 4096: W + (k + 1) * 4096]
                nc.scalar.activation(trash[0][:], sl,
                                     mybir.ActivationFunctionType.Copy,
                                     accum_out=s["acc"][:, k:k + 1])
                nc.scalar.activation(trash[1][:], sl,
                                     mybir.ActivationFunctionType.Square,
                                     accum_out=s["acc"][:, k + 4:k + 5])

            def emit_chain_bn(cc):
                # prologue path: DVE is idle before the first conv, use it
                s = st[cc]
                mv = spool.tile([128, 2], f32, tag="mv", name=f"mv{cc}")
                nc.vector.bn_aggr(mv[:], s["stats6"][:].rearrange(
                    "p (h s) -> p h s", s=6))
                stdv = spool.tile([128, 1], f32, tag="stdv", name=f"sd{cc}")
                nc.scalar.activation(stdv[:], mv[:, 1:2],
                                     mybir.ActivationFunctionType.Sqrt,
                                     scale=float(HW) / float(HW - 1))
                stde = spool.tile([128, 1], f32, tag="stde", name=f"se{cc}")
                nc.vector.tensor_scalar_add(stde[:], stdv[:], EPS)
                rstd = spool.tile([128, 1], f32, tag="rstd", name=f"rs{cc}")
                nc.vector.reciprocal(rstd[:], stde[:])
                nmr = spool.tile([128, 1], f32, tag="nmr", name=f"nm{cc}")
                nc.vector.scalar_tensor_tensor(nmr[:], mv[:, 0:1], -1.0, rstd[:],
                                               op0=MULT, op1=MULT)
                s["rstd"] = rstd
                s["nmr"] = nmr

            def emit_chain_mix(cc):
                # prologue: merge DVE bn_stats (slices 0-1) with ACT accum
                # sums (slices 2-3); all combining on the (idle) DVE
                s = st[cc]
                acc = s["acc"]
                mv = spool.tile([128, 2], f32, tag="mv", name=f"mv{cc}")
                nc.vector.bn_aggr(mv[:], s["stats6"][:, 0:96].rearrange(
                    "p (h s) -> p h s", s=6))
                s1 = spool.tile([128, 2], f32, tag="s1", name=f"s1{cc}")
                nc.vector.tensor_add(s1[:, 0:1], acc[:, 2:3], acc[:, 3:4])
                nc.vector.tensor_add(s1[:, 1:2], acc[:, 6:7], acc[:, 7:8])
                mb = spool.tile([128, 1], f32, tag="mb", name=f"mb{cc}")
                nc.vector.tensor_scalar_mul(mb[:], s1[:, 0:1], 1.0 / HW)
                mean = spool.tile([128, 1], f32, tag="mean", name=f"me{cc}")
                nc.vector.scalar_tensor_tensor(mean[:], mv[:, 0:1], 0.5, mb[:],
                                               op0=MULT, op1=ADD)
                m2a = spool.tile([128, 1], f32, tag="m2a", name=f"ma{cc}")
                nc.vector.tensor_mul(m2a[:], mv[:, 0:1], mv[:, 0:1])
                e2a = spool.tile([128, 1], f32, tag="e2a", name=f"ea{cc}")
                nc.vector.tensor_scalar(e2a[:], mv[:, 1:2], m2a[:, 0:1], 0.5,
                                        op0=ADD, op1=MULT)
                e2b = spool.tile([128, 1], f32, tag="e2b", name=f"eb{cc}")
                nc.vector.tensor_scalar_mul(e2b[:], s1[:, 1:2], 1.0 / HW)
                ex2 = spool.tile([128, 1], f32, tag="ex2", name=f"ex{cc}")
                nc.vector.tensor_add(ex2[:], e2a[:], e2b[:])
                m2 = spool.tile([128, 1], f32, tag="m2", name=f"m2{cc}")
                nc.vector.tensor_mul(m2[:], mean[:], mean[:])
                var = spool.tile([128, 1], f32, tag="var", name=f"va{cc}")
                nc.vector.tensor_sub(var[:], ex2[:], m2[:])
                stdv = spool.tile([128, 1], f32, tag="stdv", name=f"sd{cc}")
                nc.scalar.activation(stdv[:], var[:],
                                     mybir.ActivationFunctionType.Sqrt,
                                     scale=float(HW) / float(HW - 1))
                stde = spool.tile([128, 1], f32, tag="stde", name=f"se{cc}")
                nc.vector.tensor_scalar_add(stde[:], stdv[:], EPS)
                rstd = spool.tile([128, 1], f32, tag="rstd", name=f"rs{cc}")
                nc.vector.reciprocal(rstd[:], stde[:])
                nmr = spool.tile([128, 1], f32, tag="nmr", name=f"nm{cc}")
                nc.vector.scalar_tensor_tensor(nmr[:], mean[:], -1.0, rstd[:],
                                               op0=MULT, op1=MULT)
                s["rstd"] = rstd
                s["nmr"] = nmr

            def emit_chain_acc(cc):
                # chain on GpSimd (+3 ACT transcendentals): DVE stays evac-only,
                # and the gp queue has nothing PE-critical to block
                s = st[cc]
                acc = s["acc"]
                g = nc.gpsimd
                t4 = spool.tile([128, 4], f32, tag="t4", name=f"t4{cc}")
                sm = spool.tile([128, 2], f32, tag="sm", name=f"sm{cc}")
                g.tensor_add(t4[:, 0:2], acc[:, 0:2], acc[:, 2:4])
                g.tensor_add(t4[:, 2:4], acc[:, 4:6], acc[:, 6:8])
                g.tensor_add(sm[:, 0:1], t4[:, 0:1], t4[:, 1:2])
                g.tensor_add(sm[:, 1:2], t4[:, 2:3], t4[:, 3:4])
                mean = spool.tile([128, 1], f32, tag="mean", name=f"me{cc}")
                g.tensor_mul(mean[:], sm[:, 0:1], invn_sb[:])
                ex2 = spool.tile([128, 1], f32, tag="ex2", name=f"ex{cc}")
                g.tensor_mul(ex2[:], sm[:, 1:2], invn_sb[:])
                m2 = spool.tile([128, 1], f32, tag="m2", name=f"m2{cc}")
                g.tensor_mul(m2[:], mean[:], mean[:])
                var = spool.tile([128, 1], f32, tag="var", name=f"va{cc}")
                g.tensor_sub(var[:], ex2[:], m2[:])
                # rstd = 1/(sqrt(var*c)+eps) ~= exp(-0.5*ln(var*c)); the +eps
                # (1e-7 vs std~1) is a 1e-7 relative effect, far below budget
                lg = spool.tile([128, 1], f32, tag="lg", name=f"lg{cc}")
                nc.scalar.activation(lg[:], var[:],
                                     mybir.ActivationFunctionType.Ln,
                                     scale=float(HW) / float(HW - 1))
                rstd = spool.tile([128, 1], f32, tag="rstd", name=f"rs{cc}")
                nc.scalar.activation(rstd[:], lg[:],
                                     mybir.ActivationFunctionType.Exp, scale=-0.5)
                nmrp = spool.tile([128, 1], f32, tag="nmrp", name=f"np{cc}")
                g.tensor_mul(nmrp[:], mean[:], rstd[:])
                nmr = spool.tile([128, 1], f32, tag="nmr", name=f"nm{cc}")
                g.tensor_sub(nmr[:], zero_sb[:], nmrp[:])
                s["rstd"] = rstd
                s["nmr"] = nmr

            def emit_norm(cc, engines=(None,)):
                # 8 row-band pieces (conv is gated on the WHOLE normalize anyway
                # -- Tile tracks xt coarsely -- so minimize serial latency)
                s = st[cc]
                full = s["xt"][:, W:W + HW].rearrange("p (b r w) -> p b r w",
                                                      b=4, r=32)
                for q in range(8):
                    eng = engines[q % len(engines)]
                    tgt = full[:, :, 4 * q:4 * q + 4, :]
                    eng.tensor_scalar(tgt, tgt, s["rstd"][:, 0:1],
                                      s["nmr"][:, 0:1], op0=MULT, op1=ADD)

            def emit_span_mms(cc, q):
                # span q: four 64x64 array tiles = 2 channel sub-chunk PAIRS
                # (row groups R, 16 groups block-diag each) x 2 spatial halves
                # (col groups C); C covers spatial tile 16C + q
                xt = st[cc]["xt"]
                pb = [pspool.tile([128, 512], f32, tag="pb",
                                  name=f"pb{cc}_{q}_{R}") for R in range(2)]
                for ti, (dh, dwi) in enumerate(TAPS):
                    start, stop = (ti == 0), (ti == NTAP - 1)
                    tapi = dh * 3 + dwi
                    for R in range(2):
                        lhsT = w_sb[64 * R:64 * R + 64,
                                    (cc * NTAP + tapi) * 64:
                                    (cc * NTAP + tapi) * 64 + 64]
                        for Cg in range(2):
                            t = 16 * Cg + q
                            base = (4 * t + dh) * W
                            outp = pb[R][64 * Cg:64 * Cg + 64, :]
                            tp = (64 * R, 64 * Cg)
                            if dwi == 1:
                                nc.tensor.matmul(
                                    outp, lhsT,
                                    xt[64 * R:64 * R + 64, base:base + 512],
                                    start=start, stop=stop, tile_position=tp)
                            else:
                                o3 = outp.rearrange("p (h w) -> p h w", w=W)
                                r3 = xt[64 * R:64 * R + 64,
                                        base:base + 512].rearrange(
                                            "p (h w) -> p h w", w=W)
                                if dwi == 0:   # dw=-1
                                    nc.tensor.matmul(
                                        o3[:, :, 1:W], lhsT, r3[:, :, 0:W - 1],
                                        start=start, stop=stop,
                                        skip_group_check=True, tile_position=tp)
                                else:          # dw=+1
                                    nc.tensor.matmul(
                                        o3[:, :, 0:W - 1], lhsT, r3[:, :, 1:W],
                                        start=start, stop=stop,
                                        skip_group_check=True, tile_position=tp)
                return pb

            def emit_evac(cc, q, pb, om):
                for R in range(2):
                    bias_ap = bias_sb[:, cc * 2 + R: cc * 2 + R + 1]
                    dst = om[:, R * 8192 + q * 512: R * 8192 + q * 512 + 512]
                    nc.vector.tensor_scalar_add(dst, pb[R][:, :], bias_ap)

            def emit_out(cc, om, hh):
                for Cg in range(2):
                    nc.sync.dma_start(
                        out_v[cc, Cg, hh],
                        om[64 * Cg:64 * Cg + 64, :].rearrange(
                            "p (R hh e) -> p R hh e", hh=2, e=4096)[:, :, hh, :])

            # ---- prologue: chunk 0 (and chunk 1 load) up front
            # prologue: chunk 0 stats on DVE (idle), chunk 1 stats on ACT, both
            # chains done before conv(0) ends; chunk-0 norm split gp/DVE
            emit_load(0)
            emit_load(1)
            for blk in range(4):
                emit_stats_block(0, blk)
            emit_chain_bn(0)
            emit_norm(0, engines=(nc.gpsimd, nc.vector))
            for k in range(4):
                emit_stats_act(1, k)

            # steady: loads + ACT stats run two chunks ahead; chain(n) at
            # (n-1, q1) when its inputs are already done; norm(n) at (n-1, q3)
            for cc in range(NCH):
                om = opool.tile([128, 4 * 4096], bf16, tag="om", name=f"om{cc}")
                for q in range(16):
                    pb = emit_span_mms(cc, q)
                    emit_evac(cc, q, pb, om)
                    if q == 0 and cc + 2 < NCH:
                        emit_load(cc + 2)
                    if q == 1 and cc + 1 < NCH:
                        emit_chain_acc(cc + 1)
                    if q in (2, 4, 6, 8) and cc + 2 < NCH:
                        emit_stats_act(cc + 2, q // 2 - 1)
                    if q == 6 and cc + 1 < NCH:
                        emit_norm(cc + 1, engines=(nc.gpsimd,))
                    if q == 7:
                        emit_out(cc, om, 0)
                emit_out(cc, om, 1)
    nc.compile()
    return nc


def _pack_inputs(x, dw, pw, biases):
    """Host-side: fuse pw o dw, scatter into block-diag 32x32 lhsT tiles."""
    G = 128
    dwr = dw.reshape(B, G, 4, 4, 3, 3)          # [b, g, m, i, kh, kw]
    pwr = pw.reshape(B, G, 4, 4)                # [b, g, j, m]
    eff = np.einsum('bgjm,bgmikl->bgjikl', pwr, dwr)  # [b, g, j, i, kh, kw]
    # 64x64 block-diag tiles: w_host[b, 64R + 4gl + i, (cc*9+tap)*64 + 4gl + j]
    w_host = np.zeros((B, 128, NCH * NTAP * 64), dtype=np.float32)
    wv = w_host.reshape(B, 2, 16, 4, NCH, NTAP, 16, 4)  # [b,R,gl_k,i,cc,tap,gl_m,j]
    er = eff.reshape(B, NCH, 2, 16, 4, 4, NTAP)         # [b, cc, R, gl, j, i, tap]
    for gl in range(16):
        e = er[:, :, :, gl]                     # [b, cc, R, j, i, tap]
        wv[:, :, gl, :, :, :, gl, :] = e.transpose(0, 2, 4, 1, 5, 3)
    bias_host = np.zeros((B, 128, 8), dtype=np.float32)
    bfull = biases.reshape(B, C)
    p = np.arange(128)
    for cc in range(NCH):
        for R in range(2):
            bias_host[:, :, cc * 2 + R] = bfull[:, cc * 128 + 64 * R + (p % 64)]
    return w_host.astype(ml_dtypes.bfloat16), bias_host


def kernel(x, dw_kernels, pw_kernels, biases):
    from concourse.bass_utils import run_bass_kernel_spmd

    x = np.ascontiguousarray(np.asarray(x, dtype=np.float32))
    dw = np.asarray(dw_kernels, dtype=np.float32)
    pw = np.asarray(pw_kernels, dtype=np.float32)
    bs = np.asarray(biases, dtype=np.float32)

    if "nc" not in _CACHE:
        _CACHE["nc"] = _build_program()
    nc = _CACHE["nc"]

    w_host, bias_host = _pack_inputs(x, dw, pw, bs)
    xb = x.reshape(B, C, HW).astype(ml_dtypes.bfloat16)
    in_maps = [{"x": xb[i],
                "w": w_host[i],
                "bias": bias_host[i]} for i in range(B)]
    res = run_bass_kernel_spmd(nc, in_maps, core_ids=list(range(B)),
                               trace=bool(int(os.environ.get("KTRACE", "0"))))
    _CACHE["last_result"] = res
    out = np.stack([res.results[i]["out"].astype(np.float32).reshape(C, H, W)
                    for i in range(B)])
    return out


# revision 75
# speedup vs baseline: 1.3789x; 1.0301x over previous
"""AdaConv2D (instance-norm -> grouped 3x3 conv -> grouped 1x1 conv -> bias) on 8 TRN2 cores.

Strategy (pure data parallel: batch dim sharded, 1 sample per NeuronCore, no
collectives; measured ~237-245us HW exec, rel err ~2.2e-3 vs the f32 reference):

  Host-side prep (inside kernel(), before launch):
  - Fuse the grouped 1x1 conv into the grouped 3x3 conv: both share the same
    4-channel group partition, so eff[g,j,i,kh,kw] = sum_m pw[g,j,m]*dw[g,m,i,kh,kw]
    gives ONE effective grouped 3x3 conv.
  - Pack eff into block-diagonal 32x32 bf16 lhsT tiles (8 groups of 4x4 per tile),
    one per (channel sub-chunk r, tap); x is converted to bf16 (rel-err budget
    2e-2 >> bf16 rounding; halves input DMA traffic).

  Device, per 128-channel chunk (4 chunks/sample), software-pipelined 3 deep with
  STRICT engine specialization (each engine executes its scheduled queue in order,
  so cross-chunk work must never queue behind blocking predecessors):
  - Sync (HWDGE): input DMA, 4 slices/chunk, issued two chunks ahead; output DMA.
  - ACT: instance-norm statistics via Copy/Square activations with accum_out
    (sum and sum-of-squares per slice, zero DVE cost), issued two chunks ahead.
  - GpSimd: halo memsets, the mean/var/rstd chain (tensor-tensor ops + 3 ACT
    transcendentals: rstd = exp(-ln(sqrt(var*N/(N-1))+eps))), and the normalize
    pass (8 pieces, tensor_scalar x*rstd + (-mean*rstd)), one chunk ahead.
  - DVE: PSUM eviction ONLY (psum + per-partition bias -> bf16 staging tile);
    anything else on this queue stalls TensorE via PSUM back-pressure.
  - TensorE: conv as 4 concurrent 64x64 tile_position matmuls per span =
    2 channel sub-chunk PAIRS (row groups; contraction = 64 channels,
    16 groups block-diag) x 2 spatial halves (col groups).  9 taps = shifted
    APs on a row-padded SBUF layout (1 zero halo row above/below, rows of 128
    contiguous), accumulated in PSUM (start on the first dw=0 tap).  W-edge
    padding is done by SHRINKING the free dim of dw=+-1 taps (edge output
    columns simply don't receive those taps).  Spatial tile of col group C at
    span q is t = 16C + q, so each partition half owns a contiguous spatial
    half, giving 8KB-contiguous output DMA runs.  The PE is instruction-issue
    bound, so fewer/bigger tiles beat 16x 32x32 (measured 238us -> 204us);
    LDWEIGHTS is 1:1 with matmuls (walrus ldw-opt crashes, N=1024 moving
    operand is ISA-rejected for the one-PSUM-bank limit).
  - Output staged in bf16 (halves output traffic), stored per quarter-drain;
    host upcasts to f32.
"""
import os
import sys
import numpy as np
import ml_dtypes

if "/opt/trn_rl_repo" not in sys.path:
    sys.path.insert(0, "/opt/trn_rl_repo")

B, C, H, W = 8, 512, 128, 128
HW = H * W            # 16384
NCH = 4               # 128-channel chunks per sample
NTAP = 9
ROWS_PAD = H + 2      # 130 rows of 128 in padded SBUF layout
PADF = ROWS_PAD * W   # 16640 elems per partition
EPS = 1e-7
# taps ordered so the first three are dw=0 (full-width writes -> correct PSUM init)
TAPS = [(0, 1), (1, 1), (2, 1), (0, 0), (1, 0), (2, 0), (0, 2), (1, 2), (2, 2)]

_CACHE = {}


def _build_program():
    import concourse.bass as bass
    import concourse.tile as tile
    from concourse import bacc, mybir

    f32 = mybir.dt.float32
    bf16 = mybir.dt.bfloat16
    MULT = mybir.AluOpType.mult
    ADD = mybir.AluOpType.add
    SUB = mybir.AluOpType.subtract
    IDENT = mybir.ActivationFunctionType.Identity
    nc = bacc.Bacc("TRN2", target_bir_lowering=False, debug=False,
                   enable_asserts=False, num_devices=8)

    x_d = nc.dram_tensor("x", [C, HW], bf16, kind="ExternalInput")
    w_d = nc.dram_tensor("w", [128, NCH * NTAP * 64], bf16, kind="ExternalInput")
    b_d = nc.dram_tensor("bias", [128, 8], f32, kind="ExternalInput")
    out_d = nc.dram_tensor("out", [C, HW], bf16, kind="ExternalOutput")

    # store view: [cc, Ch(spatial half), hh(drain half), p, R, e(4096)]
    out_v = out_d[:].rearrange("(a R p) (Ch hh e) -> a Ch hh p R e", a=NCH, R=2,
                               p=64, Ch=2, hh=2, e=4096)

    with tile.TileContext(nc) as tc:
        with (
            tc.tile_pool(name="xpool", bufs=3) as xpool,
            tc.tile_pool(name="wpool", bufs=1) as wpool,
            tc.tile_pool(name="spool", bufs=3) as spool,
            tc.tile_pool(name="opool", bufs=2) as opool,
            tc.tile_pool(name="psum", bufs=8, space=bass.MemorySpace.PSUM) as pspool,
        ):
            w_sb = wpool.tile([128, NCH * NTAP * 64], bf16)
            nc.sync.dma_start(w_sb[:], w_d[:])
            bias_sb = wpool.tile([128, 8], f32)
            nc.sync.dma_start(bias_sb[:], b_d[:])
            trash0 = wpool.tile([128, 4096], bf16)
            trash1 = wpool.tile([128, 4096], bf16)
            trash = [trash0, trash1]
            eps_sb = wpool.tile([128, 1], f32)
            nc.gpsimd.memset(eps_sb[:], EPS)
            invn_sb = wpool.tile([128, 1], f32)
            nc.gpsimd.memset(invn_sb[:], 1.0 / HW)
            zero_sb = wpool.tile([128, 1], f32)
            nc.gpsimd.memset(zero_sb[:], 0.0)

            st = {}  # per-chunk small tiles

            def emit_load(cc):
                xt = xpool.tile([128, PADF], bf16, tag="xt", name=f"xt{cc}")
                st[cc] = {"xt": xt}
                nc.gpsimd.memset(xt[:, 0:W], 0.0)
                nc.gpsimd.memset(xt[:, PADF - W:PADF], 0.0)
                for k in range(4):
                    nc.sync.dma_start(xt[:, W + k * 4096: W + (k + 1) * 4096],
                                      x_d[cc * 128:(cc + 1) * 128,
                                          k * 4096:(k + 1) * 4096])

            def emit_stats_block(cc, blk):
                # chunk-0 (prologue) path: bn_stats on DVE
                s = st[cc]
                if blk == 0:
                    s["stats6"] = spool.tile([128, 32 * 6], f32, tag="stats",
                                             name=f"st{cc}")
                xt = s["xt"]
                for j in range(8 * blk, 8 * blk + 8):
                    nc.vector.bn_stats(s["stats6"][:, j * 6:(j + 1) * 6],
                                       xt[:, W + j * 512: W + (j + 1) * 512])

            def emit_stats_act(cc, k):
                # steady-state path: ACT accumulates sum (Copy) and sumsq (Square)
                # of DMA slice k into acc cols; zero DVE cost.
                s = st[cc]
                if "acc" not in s:
                    s["acc"] = spool.tile([128, 8], f32, tag="acc", name=f"ac{cc}")
                xt = s["xt"]
                sl = xt[:, W + k * 4096: W + (k + 1) * 4096]
                nc.scalar.activation(trash[0][:], sl,
                                     mybir.ActivationFunctionType.Copy,
                                     accum_out=s["acc"][:, k:k + 1])
                nc.scalar.activation(trash[1][:], sl,
                                     mybir.ActivationFunctionType.Square,
                                     accum_out=s["acc"][:, k + 4:k + 5])

            def emit_chain_bn(cc):
                # prologue path: DVE is idle before the first conv, use it
                s = st[cc]
                mv = spool.tile([128, 2], f32, tag="mv", name=f"mv{cc}")
                nc.vector.bn_aggr(mv[:], s["stats6"][:].rearrange(
                    "p (h s) -> p h s", s=6))
                stdv = spool.tile([128, 1], f32, tag="stdv", name=f"sd{cc}")
                nc.scalar.activation(stdv[:], mv[:, 1:2],
                                     mybir.ActivationFunctionType.Sqrt,
                                     scale=float(HW) / float(HW - 1))
                stde = spool.tile([128, 1], f32, tag="stde", name=f"se{cc}")
                nc.vector.tensor_scalar_add(stde[:], stdv[:], EPS)
                rstd = spool.tile([128, 1], f32, tag="rstd", name=f"rs{cc}")
                nc.vector.reciprocal(rstd[:], stde[:])
                nmr = spool.tile([128, 1], f32, tag="nmr", name=f"nm{cc}")
                nc.vector.scalar_tensor_tensor(nmr[:], mv[:, 0:1], -1.0, rstd[:],
                                               op0=MULT, op1=MULT)
                s["rstd"] = rstd
                s["nmr"] = nmr

            def emit_chain_mix(cc):
                # prologue: merge DVE bn_stats (slices 0-1) with ACT accum
                # sums (slices 2-3); all combining on the (idle) DVE
                s = st[cc]
                acc = s["acc"]
                mv = spool.tile([128, 2], f32, tag="mv", name=f"mv{cc}")
                nc.vector.bn_aggr(mv[:], s["stats6"][:, 0:96].rearrange(
                    "p (h s) -> p h s", s=6))
                s1 = spool.tile([128, 2], f32, tag="s1", name=f"s1{cc}")
                nc.vector.tensor_add(s1[:, 0:1], acc[:, 2:3], acc[:, 3:4])
                nc.vector.tensor_add(s1[:, 1:2], acc[:, 6:7], acc[:, 7:8])
                mb = spool.tile([128, 1], f32, tag="mb", name=f"mb{cc}")
                nc.vector.tensor_scalar_mul(mb[:], s1[:, 0:1], 1.0 / HW)
                mean = spool.tile([128, 1], f32, tag="mean", name=f"me{cc}")
                nc.vector.scalar_tensor_tensor(mean[:], mv[:, 0:1], 0.5, mb[:],
                                               op0=MULT, op1=ADD)
                m2a = spool.tile([128, 1], f32, tag="m2a", name=f"ma{cc}")
                nc.vector.tensor_mul(m2a[:], mv[:, 0:1], mv[:, 0:1])
                e2a = spool.tile([128, 1], f32, tag="e2a", name=f"ea{cc}")
                nc.vector.tensor_scalar(e2a[:], mv[:, 1:2], m2a[:, 0:1], 0.5,
                                        op0=ADD, op1=MULT)
                e2b = spool.tile([128, 1], f32, tag="e2b", name=f"eb{cc}")
                nc.vector.tensor_scalar_mul(e2b[:], s1[:, 1:2], 1.0 / HW)
                ex2 = spool.tile([128, 1], f32, tag="ex2", name=f"ex{cc}")
                nc.vector.tensor_add(ex2[:], e2a[:], e2b[:])
                m2 = spool.tile([128, 1], f32, tag="m2", name=f"m2{cc}")
                nc.vector.tensor_mul(m2[:], mean[:], mean[:])
                var = spool.tile([128, 1], f32, tag="var", name=f"va{cc}")
                nc.vector.tensor_sub(var[:], ex2[:], m2[:])
                stdv = spool.tile([128, 1], f32, tag="stdv", name=f"sd{cc}")
                nc.scalar.activation(stdv[:], var[:],
                                     mybir.ActivationFunctionType.Sqrt,
                                     scale=float(HW) / float(HW - 1))
                stde = spool.tile([128, 1], f32, tag="stde", name=f"se{cc}")
                nc.vector.tensor_scalar_add(stde[:], stdv[:], EPS)
                rstd = spool.tile([128, 1], f32, tag="rstd", name=f"rs{cc}")
                nc.vector.reciprocal(rstd[:], stde[:])
                nmr = spool.tile([128, 1], f32, tag="nmr", name=f"nm{cc}")
                nc.vector.scalar_tensor_tensor(nmr[:], mean[:], -1.0, rstd[:],
                                               op0=MULT, op1=MULT)
                s["rstd"] = rstd
                s["nmr"] = nmr

            def emit_chain_acc(cc):
                # chain on GpSimd (+3 ACT transcendentals): DVE stays evac-only,
                # and the gp queue has nothing PE-critical to block
                s = st[cc]
                acc = s["acc"]
                g = nc.gpsimd
                t4 = spool.tile([128, 4], f32, tag="t4", name=f"t4{cc}")
                sm = spool.tile([128, 2], f32, tag="sm", name=f"sm{cc}")
                g.tensor_add(t4[:, 0:2], acc[:, 0:2], acc[:, 2:4])
                g.tensor_add(t4[:, 2:4], acc[:, 4:6], acc[:, 6:8])
                g.tensor_add(sm[:, 0:1], t4[:, 0:1], t4[:, 1:2])
                g.tensor_add(sm[:, 1:2], t4[:, 2:3], t4[:, 3:4])
                mean = spool.tile([128, 1], f32, tag="mean", name=f"me{cc}")
                g.tensor_mul(mean[:], sm[:, 0:1], invn_sb[:])
                ex2 = spool.tile([128, 1], f32, tag="ex2", name=f"ex{cc}")
                g.tensor_mul(ex2[:], sm[:, 1:2], invn_sb[:])
                m2 = spool.tile([128, 1], f32, tag="m2", name=f"m2{cc}")
                g.tensor_mul(m2[:], mean[:], mean[:])
                var = spool.tile([128, 1], f32, tag="var", name=f"va{cc}")
                g.tensor_sub(var[:], ex2[:], m2[:])
                # rstd = 1/(sqrt(var*c)+eps) ~= exp(-0.5*ln(var*c)); the +eps
                # (1e-7 vs std~1) is a 1e-7 relative effect, far below budget
                lg = spool.tile([128, 1], f32, tag="lg", name=f"lg{cc}")
                nc.scalar.activation(lg[:], var[:],
                                     mybir.ActivationFunctionType.Ln,
                                     scale=float(HW) / float(HW - 1))
                rstd = spool.tile([128, 1], f32, tag="rstd", name=f"rs{cc}")
                nc.scalar.activation(rstd[:], lg[:],
                                     mybir.ActivationFunctionType.Exp, scale=-0.5)
                nmrp = spool.tile([128, 1], f32, tag="nmrp", name=f"np{cc}")
                g.tensor_mul(nmrp[:], mean[:], rstd[:])
                nmr = spool.tile([128, 1], f32, tag="nmr", name=f"nm{cc}")
                g.tensor_sub(nmr[:], zero_sb[:], nmrp[:])
                s["rstd"] = rstd
                s["nmr"] = nmr

            def emit_norm(cc, engines=(None,)):
                # 8 row-band pieces (conv is gated on the WHOLE normalize anyway
                # -- Tile tracks xt coarsely -- so minimize serial latency)
                s = st[cc]
                full = s["xt"][:, W:W + HW].rearrange("p (b r w) -> p b r w",
                                                      b=4, r=32)
                for q in range(8):
                    eng = engines[q % len(engines)]
                    tgt = full[:, :, 4 * q:4 * q + 4, :]
                    eng.tensor_scalar(tgt, tgt, s["rstd"][:, 0:1],
                                      s["nmr"][:, 0:1], op0=MULT, op1=ADD)

            def emit_span_mms(cc, q):
                # span q: four 64x64 array tiles = 2 channel sub-chunk PAIRS
                # (row groups R, 16 groups block-diag each) x 2 spatial halves
                # (col groups C); C covers spatial tile 16C + q
                xt = st[cc]["xt"]
                pb = [pspool.tile([128, 512], f32, tag="pb",
                                  name=f"pb{cc}_{q}_{R}") for R in range(2)]
                for ti, (dh, dwi) in enumerate(TAPS):
                    start, stop = (ti == 0), (ti == NTAP - 1)
                    tapi = dh * 3 + dwi
                    for R in range(2):
                        lhsT = w_sb[64 * R:64 * R + 64,
                                    (cc * NTAP + tapi) * 64:
                                    (cc * NTAP + tapi) * 64 + 64]
                        for Cg in range(2):
                            t = 16 * Cg + q
                            base = (4 * t + dh) * W
                            outp = pb[R][64 * Cg:64 * Cg + 64, :]
                            tp = (64 * R, 64 * Cg)
                            if dwi == 1:
                                nc.tensor.matmul(
                                    outp, lhsT,
                                    xt[64 * R:64 * R + 64, base:base + 512],
                                    start=start, stop=stop, tile_position=tp)
                            else:
                                o3 = outp.rearrange("p (h w) -> p h w", w=W)
                                r3 = xt[64 * R:64 * R + 64,
                                        base:base + 512].rearrange(
                                            "p (h w) -> p h w", w=W)
                                if dwi == 0:   # dw=-1
                                    nc.tensor.matmul(
                                        o3[:, :, 1:W], lhsT, r3[:, :, 0:W - 1],
                                        start=start, stop=stop,
                                        skip_group_check=True, tile_position=tp)
                                else:          # dw=+1
                                    nc.tensor.matmul(
                                        o3[:, :, 0:W - 1], lhsT, r3[:, :, 1:W],
                                        start=start, stop=stop,
                                        skip_group_check=True, tile_position=tp)
                return pb

            def emit_evac(cc, q, pb, om):
                for R in range(2):
                    bias_ap = bias_sb[:, cc * 2 + R: cc * 2 + R + 1]
                    dst = om[:, R * 8192 + q * 512: R * 8192 + q * 512 + 512]
                    nc.vector.tensor_scalar_add(dst, pb[R][:, :], bias_ap)

            def emit_out(cc, om, hh):
                for Cg in range(2):
                    nc.sync.dma_start(
                        out_v[cc, Cg, hh],
                        om[64 * Cg:64 * Cg + 64, :].rearrange(
                            "p (R hh e) -> p R hh e", hh=2, e=4096)[:, :, hh, :])

            # finer store view for the last chunk's drains (shrinks the tail)
            out_v4 = out_d[:].rearrange("(a R p) (Ch qq e) -> a Ch qq p R e",
                                        a=NCH, R=2, p=64, Ch=2, qq=4, e=2048)

            def emit_out4(cc, om, part):
                for Cg in range(2):
                    nc.sync.dma_start(
                        out_v4[cc, Cg, part],
                        om[64 * Cg:64 * Cg + 64, :].rearrange(
                            "p (R qq e) -> p R qq e", qq=4, e=2048)[:, :, part, :])

            # ---- prologue: chunk 0 (and chunk 1 load) up front
            # prologue: chunk 0 stats on DVE (idle), chunk 1 stats on ACT, both
            # chains done before conv(0) ends; chunk-0 norm split gp/DVE
            emit_load(0)
            emit_load(1)
            for blk in range(4):
                emit_stats_block(0, blk)
            emit_chain_bn(0)
            emit_norm(0, engines=(nc.gpsimd, nc.vector))
            for k in range(4):
                emit_stats_act(1, k)

            # steady: loads + ACT stats run two chunks ahead; chain(n) at
            # (n-1, q1) when its inputs are already done; norm(n) at (n-1, q3)
            for cc in range(NCH):
                om = opool.tile([128, 4 * 4096], bf16, tag="om", name=f"om{cc}")
                for q in range(16):
                    pb = emit_span_mms(cc, q)
                    emit_evac(cc, q, pb, om)
                    if q == 0 and cc + 2 < NCH:
                        emit_load(cc + 2)
                    if q == 1 and cc + 1 < NCH:
                        emit_chain_acc(cc + 1)
                    if q in (2, 4, 6, 8) and cc + 2 < NCH:
                        emit_stats_act(cc + 2, q // 2 - 1)
                    if q == 6 and cc + 1 < NCH:
                        emit_norm(cc + 1, engines=(nc.gpsimd,))
                    if cc < NCH - 1:
                        if q == 7:
                            emit_out(cc, om, 0)
                    elif q in (3, 7, 11):
                        emit_out4(cc, om, q // 4)
                if cc < NCH - 1:
                    emit_out(cc, om, 1)
                else:
                    emit_out4(cc, om, 3)
    nc.compile()
    return nc


def _pack_inputs(x, dw, pw, biases):
    """Host-side: fuse pw o dw, scatter into block-diag 32x32 lhsT tiles."""
    G = 128
    dwr = dw.reshape(B, G, 4, 4, 3, 3)          # [b, g, m, i, kh, kw]
    pwr = pw.reshape(B, G, 4, 4)                # [b, g, j, m]
    eff = np.einsum('bgjm,bgmikl->bgjikl', pwr, dwr)  # [b, g, j, i, kh, kw]
    # 64x64 block-diag tiles: w_host[b, 64R + 4gl + i, (cc*9+tap)*64 + 4gl + j]
    w_host = np.zeros((B, 128, NCH * NTAP * 64), dtype=np.float32)
    wv = w_host.reshape(B, 2, 16, 4, NCH, NTAP, 16, 4)  # [b,R,gl_k,i,cc,tap,gl_m,j]
    er = eff.reshape(B, NCH, 2, 16, 4, 4, NTAP)         # [b, cc, R, gl, j, i, tap]
    for gl in range(16):
        e = er[:, :, :, gl]                     # [b, cc, R, j, i, tap]
        wv[:, :, gl, :, :, :, gl, :] = e.transpose(0, 2, 4, 1, 5, 3)
    bias_host = np.zeros((B, 128, 8), dtype=np.float32)
    bfull = biases.reshape(B, C)
    p = np.arange(128)
    for cc in range(NCH):
        for R in range(2):
            bias_host[:, :, cc * 2 + R] = bfull[:, cc * 128 + 64 * R + (p % 64)]
    return w_host.astype(ml_dtypes.bfloat16), bias_host


def kernel(x, dw_kernels, pw_kernels, biases):
    from concourse.bass_utils import run_bass_kernel_spmd

    x = np.ascontiguousarray(np.asarray(x, dtype=np.float32))
    dw = np.asarray(dw_kernels, dtype=np.float32)
    pw = np.asarray(pw_kernels, dtype=np.float32)
    bs = np.asarray(biases, dtype=np.float32)

    if "nc" not in _CACHE:
        _CACHE["nc"] = _build_program()
    nc = _CACHE["nc"]

    w_host, bias_host = _pack_inputs(x, dw, pw, bs)
    xb = x.reshape(B, C, HW).astype(ml_dtypes.bfloat16)
    in_maps = [{"x": xb[i],
                "w": w_host[i],
                "bias": bias_host[i]} for i in range(B)]
    res = run_bass_kernel_spmd(nc, in_maps, core_ids=list(range(B)),
                               trace=bool(int(os.environ.get("KTRACE", "0"))))
    _CACHE["last_result"] = res
    out = np.stack([res.results[i]["out"].astype(np.float32).reshape(C, H, W)
                    for i in range(B)])
    return out
